# revision 8
# baseline (speedup 1.0000x reference)
"""DH-SRNN (dendritic-heterogeneity spiking RNN) forward on 8 Trainium2 cores.

Data-parallel over batch (B=256 -> 32 rows/core), weights replicated.

Math restructuring (host-side, exact):
  beta = sigmoid(tau_n)[H,BR], alpha = sigmoid(tau_m)[H], aro = sigmoid(tau_m_ro)[O]
  features permuted branch-major: f' = br*H + h
  fold c[f'] = (1-alpha[h])*(1-beta[h,br]) into W_dense rows/bias, so with
  D := (1-alpha)*d_in (optionally scaled by WH_SCALE for fp8 weights):
     D_t   = beta*D_{t-1} + (xp'_t + spk_{t-1} @ Wh'^T)
     mem_t = alpha*mem_{t-1} + sum_br D_t - s*spk_{t-1}
     spk_t = (mem_t > s)                      (s = WH_SCALE if fp8 else 1)
  xp'_t = x_t @ Wx'^T + b'  precomputed on-device for all t (bias via x-row==1).
  readout (transposed layout [O, BL], per-step; softmax DEFERRED):
     mem_roT_t = aro*mem_roT_{t-1} + Wro' @ spk_t + bro'
     store mem_roT_t (bf16) into a history buffer
  end-pass: out = sum_{t>10} softmax(mem_roT_t) computed chunk-wise with
  PE column-sums + PE broadcast of 1/sum, all off the critical scan path.

Device layouts (per core, BL=32 batch rows):
  f' blocks jf=0..31 (f' = jf*128+p), h blocks jh=0..7 (h = jh*128+p)
  d    SBUF [128, jf*32+b] f32      mem/spk SBUF [128, jh*32+b]
  whT  SBUF [128, (jhk*32+jf)*128+m] fp8e4m3 x 2^13 (lhsT tiles)
  xp   DRAM [128, chunk-major: c*(JF*512) + jf*512 + (s*32+b)] bf16
       (chunk = 16 timesteps; phase-1 stores and scan loads are contiguous)
"""

import numpy as np
import ml_dtypes

import concourse.bass as bass
import concourse.bacc as bacc
import concourse.mybir as mybir
import concourse.tile as tile
from concourse.bass_utils import run_bass_kernel_spmd

F32 = mybir.dt.float32
BF16 = mybir.dt.bfloat16
FP8 = mybir.dt.float8e4

B, T_FULL, IN_DIM = 256, 500, 700
H, BR, O = 1024, 4, 20
NCORES = 8
BL = B // NCORES            # 32 batch rows per core
KT = 6                      # k-tiles for input dim (700 + bias row -> 768)
KIN = KT * 128              # 768
JF = (H * BR) // 128        # 32 feature blocks
JH = H // 128               # 8 hidden blocks
CH = 16                     # timesteps per xp chunk (512 cols = 1 psum bank)
NCH = CH * BL               # 512 columns per chunk
CHW = JF * NCH              # xp cols per chunk (16384)
WARMUP = 10
WH_SCALE = 8192.0           # 2**13 centers Wh in fp8e4m3's normal range


def _sigmoid(x):
    return 1.0 / (1.0 + np.exp(-x))


def _bf(a):
    return np.ascontiguousarray(a.astype(ml_dtypes.bfloat16))


def _f32(a):
    return np.ascontiguousarray(a.astype(np.float32))


def _fp8(a):
    return np.ascontiguousarray(
        np.clip(a, -448.0, 448.0).astype(ml_dtypes.float8_e4m3))


def prepare_inputs(x, W_dense, b_dense, tau_n, tau_m, W_ro, b_ro, tau_m_ro,
                   wh_fp8=True):
    x = np.asarray(x, np.float32)
    W = np.asarray(W_dense, np.float32)
    b = np.asarray(b_dense, np.float32)
    beta = _sigmoid(np.asarray(tau_n, np.float32))      # [H, BR]
    alpha = _sigmoid(np.asarray(tau_m, np.float32))     # [H]
    aro = _sigmoid(np.asarray(tau_m_ro, np.float32))    # [O]
    W_ro = np.asarray(W_ro, np.float32)
    b_ro = np.asarray(b_ro, np.float32)

    # branch-major permutation f' = br*H + h  (row f = h*BR + br)
    brs, hs = np.meshgrid(np.arange(BR), np.arange(H), indexing="ij")
    perm = (hs * BR + brs).reshape(-1)
    Wp = W[perm]                                         # [4096, 1724]
    bp = b[perm]
    beta_f = beta.T.reshape(-1)                          # beta[f'=br*H+h]
    alpha_f = np.tile(alpha, BR)                         # alpha[h] per f'
    c = (1.0 - alpha_f) * (1.0 - beta_f)

    Wx = c[:, None] * Wp[:, :IN_DIM]                     # [4096, 700]
    Wh = c[:, None] * Wp[:, IN_DIM:]                     # [4096, 1024]
    bp = c * bp

    if wh_fp8:
        Wx = Wx * WH_SCALE
        Wh = Wh * WH_SCALE
        bp = bp * WH_SCALE

    Wx_aug = np.zeros((H * BR, KIN), np.float32)
    Wx_aug[:, :IN_DIM] = Wx
    Wx_aug[:, IN_DIM] = bp                               # bias via x-row == 1

    # lhsT packs: [p, (kt|jhk, jf), m] with lhsT[p, m] = W[jf*128+m, kt*128+p]
    wxT = Wx_aug.reshape(JF, 128, KT, 128).transpose(3, 2, 0, 1).reshape(128, KT * JF * 128)
    whT = Wh.reshape(JF, 128, JH, 128).transpose(3, 2, 0, 1).reshape(128, JH * JF * 128)

    beta_sb = np.repeat(beta_f.reshape(JF, 128).T[:, :, None], BL, axis=2).reshape(128, JF * BL)
    alpha_sb = np.repeat(alpha.reshape(JH, 128).T[:, :, None], BL, axis=2).reshape(128, JH * BL)

    Wrop = (1.0 - aro)[:, None] * W_ro                   # [O, H]
    brop = (1.0 - aro) * b_ro
    wroT = Wrop.reshape(O, JH, 128).transpose(2, 1, 0).reshape(128, JH * O)

    common = {
        "whT": _fp8(whT) if wh_fp8 else _bf(whT),
        "wxT": _bf(wxT),
        "beta": _f32(beta_sb),
        "alpha": _f32(alpha_sb),
        "wro": _bf(wroT),
        "bro": _bf(brop.reshape(1, O)),
        "aroc": _f32(aro.reshape(O, 1)),
    }

    n_chk = (x.shape[1] * BL + NCH - 1) // NCH
    in_maps = []
    for core in range(NCORES):
        xc = x[core * BL:(core + 1) * BL]                # [32, T, 700]
        t_len = xc.shape[1]
        xT = np.zeros((KIN, n_chk * NCH), np.float32)    # zero pad past t_len
        xT[:IN_DIM, :t_len * BL] = xc.transpose(2, 1, 0).reshape(IN_DIM, t_len * BL)
        xT[IN_DIM, :t_len * BL] = 1.0
        m = dict(common)
        m["xT"] = _bf(xT)
        in_maps.append(m)
    return in_maps


def build_module(t_len=T_FULL, repeat=1, pre_mult=1, scan_mult=1,
                 phases="both", wh_fp8=True, readout=True, pool_ops=False):
    # scan structure: chunk 0 peeled, steady chunk pairs, short tail chunk
    n_chk = (t_len * BL + NCH - 1) // NCH                # xp chunks (32 for T=500)
    tail_steps = t_len - (n_chk - 1) * CH                # steps in last chunk
    assert n_chk >= 4 and (n_chk - 2) % 2 == 0, \
        "steady loop needs an even number of full chunks after the peel"

    nc = bacc.Bacc("TRN2", target_bir_lowering=False, debug=False)

    xT = nc.dram_tensor("xT", [KIN, n_chk * NCH], BF16, kind="ExternalInput").ap()
    whT = nc.dram_tensor("whT", [128, JH * JF * 128],
                         FP8 if wh_fp8 else BF16, kind="ExternalInput").ap()
    wxT = nc.dram_tensor("wxT", [128, KT * JF * 128], BF16, kind="ExternalInput").ap()
    beta_in = nc.dram_tensor("beta", [128, JF * BL], F32, kind="ExternalInput").ap()
    alpha_in = nc.dram_tensor("alpha", [128, JH * BL], F32, kind="ExternalInput").ap()
    wro_in = nc.dram_tensor("wro", [128, JH * O], BF16, kind="ExternalInput").ap()
    bro_in = nc.dram_tensor("bro", [1, O], BF16, kind="ExternalInput").ap()
    aroc_in = nc.dram_tensor("aroc", [O, 1], F32, kind="ExternalInput").ap()
    out = nc.dram_tensor("out", [O, BL], F32, kind="ExternalOutput").ap()
    xp = nc.dram_tensor("xp", [128, n_chk * CHW], BF16).ap()
    hist = nc.dram_tensor("hist", [O, n_chk * CHW], BF16).ap()

    with tile.TileContext(nc) as tc:
        _emit(tc, xT, whT, wxT, beta_in, alpha_in, wro_in, bro_in, aroc_in,
              out, xp, hist, n_chk=n_chk, tail_steps=tail_steps, repeat=repeat,
              pre_mult=pre_mult, scan_mult=scan_mult, phases=phases,
              wh_fp8=wh_fp8, readout=readout, pool_ops=pool_ops)
    nc.compile()
    return nc


def _emit(tc, xT, whT, wxT, beta_in, alpha_in, wro_in, bro_in, aroc_in,
          out, xp, hist, n_chk, tail_steps, repeat=1, pre_mult=1, scan_mult=1,
          phases="both", wh_fp8=True, readout=True, pool_ops=False):
    nc = tc.nc
    do_pre = phases in ("both", "pre")
    do_scan = phases in ("both", "scan")
    vth = WH_SCALE if wh_fp8 else 1.0

    SCR = 2 * CHW                                        # scratch cols (bf16)

    with (
        tc.tile_pool(name="const", bufs=1) as cpool,
        tc.tile_pool(name="state", bufs=1) as spool,
        tc.tile_pool(name="sm", bufs=2) as smp,
        tc.tile_pool(name="mmps", bufs=4, space="PSUM") as mmps,
        tc.tile_pool(name="rops", bufs=4, space="PSUM") as rops,
    ):
        # ---- resident constants ----
        wx_sb = cpool.tile([128, KT * JF * 128], BF16, tag="wx")
        wh_sb = cpool.tile([128, JH * JF * 128], FP8 if wh_fp8 else BF16, tag="wh")
        beta_sb = cpool.tile([128, JF * BL], F32, tag="beta")
        alpha_sb = cpool.tile([128, JH * BL], F32, tag="alpha")
        wro_sb = cpool.tile([128, JH * O], BF16, tag="wro")
        bro_sb = cpool.tile([1, O], BF16, tag="bro")
        aroc_sb = cpool.tile([O, 1], F32, tag="aroc")
        ones_sb = cpool.tile([1, BL], BF16, tag="ones")
        ones20 = cpool.tile([O, 1], F32, tag="ones20")
        onesr = cpool.tile([1, O], F32, tag="onesr")
        nc.sync.dma_start(wx_sb[:], wxT[:])
        nc.sync.dma_start(wh_sb[:], whT[:])
        nc.sync.dma_start(beta_sb[:], beta_in[:])
        nc.sync.dma_start(alpha_sb[:], alpha_in[:])
        nc.sync.dma_start(wro_sb[:], wro_in[:])
        nc.sync.dma_start(bro_sb[:], bro_in[:])
        nc.sync.dma_start(aroc_sb[:], aroc_in[:])
        nc.vector.memset(ones_sb[:], 1.0)
        nc.vector.memset(ones20[:], 1.0)
        nc.vector.memset(onesr[:], 1.0)

        # ---- scratch: phase-1 staging / scan chunk buffers / end-pass ----
        scratch = spool.tile([128, SCR], BF16, tag="scratch")
        xin = [[scratch[:, (i * KT + kt) * NCH:(i * KT + kt + 1) * NCH]
                for kt in range(KT)] for i in range(2)]
        evb = [scratch[:, 2 * KT * NCH + i * 8 * NCH:
                       2 * KT * NCH + (i + 1) * 8 * NCH] for i in range(2)]
        xpc = [scratch[:, i * CHW:(i + 1) * CHW] for i in range(2)]

        # ---- persistent state ----
        d = spool.tile([128, JF * BL], F32, tag="d")
        mem = spool.tile([128, JH * BL], F32, tag="mem")
        spk = [spool.tile([128, JH * BL], BF16, tag=f"spk{i}", name=f"spk{i}")
               for i in range(2)]
        spkb = ([spool.tile([128, JH * BL], F32, tag=f"spkb{i}", name=f"spkb{i}")
                 for i in range(2)] if wh_fp8 else spk)
        mem_roT = spool.tile([O, BL], F32, tag="mrt")
        accT = spool.tile([O, BL], F32, tag="accT")
        histc = [spool.tile([O, NCH], BF16, tag=f"hc{i}", name=f"hc{i}")
                 for i in range(2)]
        l_t1 = spool.tile([128, 4 * BL], F32, tag="lt1")
        l_t2 = spool.tile([128, 4 * BL], F32, tag="lt2")
        l_half = spool.tile([128, 4 * BL], F32, tag="lh")

        # ================= phase 1: xp = x @ Wx'^T =================
        def load_x(buf, c):
            for kt in range(KT):
                nc.sync.dma_start(xin[buf][kt],
                                  xT[kt * 128:(kt + 1) * 128, c * NCH:(c + 1) * NCH])

        def pre_chunk(buf, c):
            for g in range(4):                           # 4 groups x 8 jf-blocks
                ev = evb[g % 2]
                for jl in range(8):
                    jf = g * 8 + jl
                    ps = mmps.tile([128, 512], F32, tag="mm")
                    for kt in range(KT):
                        nc.tensor.matmul(
                            ps[:],
                            wx_sb[:, (kt * JF + jf) * 128:(kt * JF + jf + 1) * 128],
                            xin[buf][kt],
                            start=(kt == 0), stop=(kt == KT - 1),
                        )
                    nc.scalar.copy(ev[:, jl * NCH:(jl + 1) * NCH], ps[:])
                nc.scalar.dma_start(
                    xp[:, c * CHW + g * 8 * NCH: c * CHW + (g + 1) * 8 * NCH],
                    ev)

        # ================= phase 2: the scan =================
        def load_chunk(buf, xoff):
            """DMA xp chunk (16 steps, contiguous CHW cols) into xpc[buf]."""
            src = (xp[:, bass.ds(xoff, CHW)] if not isinstance(xoff, int)
                   else xp[:, xoff:xoff + CHW])
            nc.sync.dma_start(xpc[buf], src)

        d_v = d[:].rearrange("p (br c) -> p br c", br=BR)
        beta_v = beta_sb[:].rearrange("p (br c) -> p br c", br=BR)

        def emit_step(buf, s, par):
            """One timestep: s = index within chunk, par = parity of global t."""
            spk_prev, spk_cur = spk[par ^ 1], spk[par]
            xpc_v = xpc[buf].rearrange("p (br q s b) -> p br q s b",
                                       br=BR, q=JH, s=CH)
            hps = []
            for half in (0, 1):
                hp = mmps.tile([128, 512], F32, tag="mm")
                for bri in range(BR):
                    for jho in range(4):
                        jf = bri * 8 + half * 4 + jho
                        o_sl = hp[:, bri * 128 + jho * 32: bri * 128 + jho * 32 + 32]
                        for jhk in range(JH):
                            nc.tensor.matmul(
                                o_sl,
                                wh_sb[:, (jhk * JF + jf) * 128:(jhk * JF + jf + 1) * 128],
                                spk_prev[:, jhk * 32:jhk * 32 + 32],
                                start=(jhk == 0), stop=(jhk == JH - 1),
                            )
                hps.append(hp)

            for half in (0, 1):
                hp = hps[half]
                off = half * 128
                dsl = d_v[:, :, off:off + 128]
                bsl = beta_v[:, :, off:off + 128]
                xsl = xpc_v[:, :, half * 4:half * 4 + 4, s, :]
                nc.vector.tensor_tensor(dsl, dsl, bsl, mybir.AluOpType.mult)
                nc.vector.tensor_tensor(dsl, dsl, hp[:], mybir.AluOpType.add)
                nc.vector.tensor_tensor(dsl, dsl, xsl, mybir.AluOpType.add)
                # branch sum -> l_half [128, 128] (SBUF-only: Pool-eligible)
                eng = nc.gpsimd if pool_ops else nc.vector
                eng.tensor_tensor(l_t1[:], d_v[:, 0, off:off + 128],
                                  d_v[:, 1, off:off + 128], mybir.AluOpType.add)
                eng.tensor_tensor(l_t2[:], d_v[:, 2, off:off + 128],
                                  d_v[:, 3, off:off + 128], mybir.AluOpType.add)
                eng.tensor_tensor(l_half[:], l_t1[:], l_t2[:], mybir.AluOpType.add)
                # mem update + spike
                msl = mem[:, off:off + 128]
                eng.tensor_tensor(msl, msl, alpha_sb[:, off:off + 128],
                                  mybir.AluOpType.mult)
                eng.tensor_tensor(msl, msl, l_half[:], mybir.AluOpType.add)
                eng.tensor_tensor(msl, msl, spkb[par ^ 1][:, off:off + 128],
                                  mybir.AluOpType.subtract)
                eng.tensor_scalar(spk_cur[:, off:off + 128], msl, vth, None,
                                  mybir.AluOpType.is_gt)
                if wh_fp8:
                    eng.tensor_scalar(spkb[par][:, off:off + 128], msl, vth,
                                      vth, mybir.AluOpType.is_gt,
                                      mybir.AluOpType.mult)

            # ---- readout: transposed [O, BL]; softmax deferred to end-pass ----
            if not readout:
                return
            roT = rops.tile([O, BL], F32, tag="ro")
            for jh in range(JH):
                nc.tensor.matmul(
                    roT[:], wro_sb[:, jh * O:(jh + 1) * O],
                    spk_cur[:, jh * 32:jh * 32 + 32],
                    start=(jh == 0), stop=False,
                )
            nc.tensor.matmul(roT[:], bro_sb[:], ones_sb[:], start=False, stop=True)
            nc.scalar.activation(mem_roT[:], mem_roT[:],
                                 mybir.ActivationFunctionType.Copy,
                                 scale=aroc_sb[:, 0:1])
            nc.vector.tensor_tensor(mem_roT[:], mem_roT[:], roT[:],
                                    mybir.AluOpType.add)
            nc.scalar.copy(histc[buf][:, s * BL:(s + 1) * BL], mem_roT[:])

        def flush_hist(buf, xoff, ncols=NCH):
            if not readout:
                return
            dst = (hist[:, bass.ds(xoff, ncols)] if not isinstance(xoff, int)
                   else hist[:, xoff:xoff + ncols])
            nc.scalar.dma_start(dst, histc[buf][:, :ncols])

        for _rep in range(repeat):
            # ---- reset state ----
            nc.vector.memset(d[:], 0.0)
            nc.vector.memset(mem[:], 0.0)
            nc.vector.memset(spk[0][:], 0.0)
            nc.vector.memset(spk[1][:], 0.0)
            if wh_fp8:
                nc.vector.memset(spkb[0][:], 0.0)
                nc.vector.memset(spkb[1][:], 0.0)
            nc.vector.memset(mem_roT[:], 0.0)
            nc.vector.memset(accT[:], 0.0)

            # ---- phase 1 (fully unrolled; loads on qSP, stores on qAct) ----
            if do_pre:
                for _pm in range(pre_mult):
                    load_x(0, 0)
                    for c in range(n_chk):
                        if c + 1 < n_chk:
                            load_x((c + 1) % 2, c + 1)
                        pre_chunk(c % 2, c)

            tc.strict_bb_all_engine_barrier()

            # ---- phase 2: scan ----
            if do_scan:
                for _sm in range(scan_mult):
                    # peel: chunk 0 (t = 0..CH-1), prefetch chunk 1
                    load_chunk(0, 0)
                    load_chunk(1, CHW)
                    for s in range(CH):
                        emit_step(0, s, s & 1)
                    flush_hist(0, 0)

                    # steady: chunk pairs (1,2), (3,4), ... (n_chk-3, n_chk-2)
                    with tc.For_i(CHW, (n_chk - 2) * CHW, 2 * CHW,
                                  hint_engines=(mybir.EngineType.PE,)) as xoff:
                        load_chunk(0, xoff + CHW)
                        for s in range(CH):
                            emit_step(1, s, s & 1)
                        flush_hist(1, xoff)
                        load_chunk(1, xoff + 2 * CHW)     # prefetch (last: tail chunk)
                        for s in range(CH):
                            emit_step(0, s, s & 1)
                        flush_hist(0, xoff + CHW)

                    # tail: chunk n_chk-1 (already in xpc[1]); flush all NCH
                    # cols (stale cols stay finite and are excluded from the
                    # end-pass reduce)
                    for s in range(tail_steps):
                        emit_step(1, s, s & 1)
                    flush_hist(1, (n_chk - 1) * CHW)

            # ---- end-pass: out = sum_{t>WARMUP} softmax(mem_roT_t) ----
            if do_scan and readout:
                hist_sb = scratch[0:O, 0:n_chk * NCH]
                hist_v = hist_sb.rearrange("p (c n) -> p c n", c=n_chk)
                src = hist.rearrange("p (c w) -> p c w", c=n_chk)
                nc.sync.dma_start(hist_v, src[:, :, 0:NCH])
                for c in range(n_chk):
                    t0 = c * CH
                    lo = max(WARMUP + 1 - t0, 0)          # skip warmup steps
                    hi = tail_steps if c == n_chk - 1 else CH
                    if lo >= hi:
                        continue
                    hv = hist_v[:, c, :]
                    e = smp.tile([O, NCH], F32, tag="e")
                    nc.scalar.activation(e[:], hv, mybir.ActivationFunctionType.Exp)
                    sp = mmps.tile([1, 512], F32, tag="mm")
                    nc.tensor.matmul(sp[:, :NCH], ones20[:], e[:], start=True,
                                     stop=True)
                    rp = smp.tile([1, NCH], F32, tag="rp")
                    nc.vector.reciprocal(rp[:], sp[:, :NCH])
                    bc = mmps.tile([128, 512], F32, tag="mm")
                    nc.tensor.matmul(bc[:O, :NCH], onesr[:], rp[:], start=True,
                                     stop=True)
                    pr = smp.tile([O, NCH], F32, tag="pr")
                    nc.vector.tensor_tensor(pr[:], e[:], bc[:O, :NCH],
                                            mybir.AluOpType.mult)
                    pr_v = pr[:].rearrange("p (s b) -> p b s", s=CH)
                    red = smp.tile([O, BL], F32, tag="red")
                    nc.vector.tensor_reduce(red[:], pr_v[:, :, lo:hi],
                                            mybir.AxisListType.X,
                                            mybir.AluOpType.add)
                    nc.vector.tensor_tensor(accT[:], accT[:], red[:],
                                            mybir.AluOpType.add)

            nc.sync.dma_start(out[:], accT[:])


_NC_CACHE = {}


def _get_module(t_len):
    if t_len not in _NC_CACHE:
        _NC_CACHE[t_len] = build_module(t_len)
    return _NC_CACHE[t_len]


def run(inputs, trace=False):
    in_maps = prepare_inputs(**inputs)
    t_len = np.asarray(inputs["x"]).shape[1]
    nc = _get_module(t_len)
    res = run_bass_kernel_spmd(nc, in_maps, list(range(NCORES)), trace=trace)
    outs = [res.results[i]["out"].T for i in range(NCORES)]   # [O,BL] -> [BL,O]
    return np.concatenate(outs, axis=0).astype(np.float32), res


def kernel(x, W_dense, b_dense, tau_n, tau_m, W_ro, b_ro, tau_m_ro):
    out, _ = run(dict(x=x, W_dense=W_dense, b_dense=b_dense, tau_n=tau_n,
                      tau_m=tau_m, W_ro=W_ro, b_ro=b_ro, tau_m_ro=tau_m_ro))
    return out


def make_bench(inputs, nc=None, prep_kwargs=None):
    """Build a timed runner with device-resident inputs (for test.py only).

    Mirrors bass2jax.run_bass_via_pjrt's multi-core path, but device_puts the
    inputs once so repeated calls measure device execution, not host transfer.
    """
    import jax
    import numpy as np_
    from jax.sharding import Mesh, PartitionSpec, NamedSharding
    from jax.experimental.shard_map import shard_map
    import concourse.mybir as mybir_
    from concourse import bass2jax

    in_maps = prepare_inputs(**inputs, **(prep_kwargs or {}))
    t_len = np_.asarray(inputs["x"]).shape[1]
    if nc is None:
        nc = _get_module(t_len)
    bass2jax.install_neuronx_cc_hook()

    partition_name = nc.partition_id_tensor.name if nc.partition_id_tensor else None
    in_names, out_names, out_avals, zero_outs = [], [], [], []
    for alloc in nc.m.functions[0].allocations:
        if not isinstance(alloc, mybir_.MemoryLocationSet):
            continue
        name = alloc.memorylocations[0].name
        if alloc.kind == "ExternalInput":
            if name != partition_name:
                in_names.append(name)
        elif alloc.kind == "ExternalOutput":
            shape = tuple(alloc.tensor_shape)
            dtype = mybir_.dt.np(alloc.dtype)
            out_names.append(name)
            out_avals.append(jax.core.ShapedArray(shape, dtype))
            zero_outs.append(np_.zeros(shape, dtype))
    n_params = len(in_names)
    all_in_names = in_names + out_names
    if partition_name is not None:
        all_in_names.append(partition_name)
    donate = tuple(range(n_params, n_params + len(out_names)))

    def _body(*args):
        operands = list(args)
        if partition_name is not None:
            operands.append(bass2jax.partition_id_tensor())
        outs = bass2jax._bass_exec_p.bind(
            *operands,
            out_avals=tuple(out_avals),
            in_names=tuple(all_in_names),
            out_names=tuple(out_names),
            lowering_input_output_aliases=(),
            sim_require_finite=True,
            sim_require_nnan=True,
            nc=nc,
        )
        return tuple(outs)

    devices = jax.devices()[:NCORES]
    mesh = Mesh(np_.asarray(devices), ("core",))
    in_specs = (PartitionSpec("core"),) * (n_params + len(out_names))
    out_specs = (PartitionSpec("core"),) * len(out_names)
    sharded = jax.jit(
        shard_map(_body, mesh=mesh, in_specs=in_specs, out_specs=out_specs,
                  check_rep=False),
        donate_argnums=donate, keep_unused=True,
    )
    concat_in = [
        np_.concatenate([np_.asarray(in_maps[c][name]) for c in range(NCORES)], axis=0)
        for name in in_names
    ]
    sh = NamedSharding(mesh, PartitionSpec("core"))
    dev_in = [jax.device_put(a, sh) for a in concat_in]

    def call():
        zeros = [np_.zeros((NCORES * z.shape[0], *z.shape[1:]), z.dtype)
                 for z in zero_outs]
        outs = sharded(*dev_in, *zeros)
        jax.block_until_ready(outs)
        return outs

    return call


# revision 9
# speedup vs baseline: 1.0389x; 1.0389x over previous
"""DH-SRNN (dendritic-heterogeneity spiking RNN) forward on 8 Trainium2 cores.

Data-parallel over batch (B=256 -> 32 rows/core), weights replicated.

Math restructuring (host-side, exact):
  beta = sigmoid(tau_n)[H,BR], alpha = sigmoid(tau_m)[H], aro = sigmoid(tau_m_ro)[O]
  features permuted branch-major: f' = br*H + h
  fold c[f'] = (1-alpha[h])*(1-beta[h,br]) into W_dense rows/bias, so with
  D := (1-alpha)*d_in (optionally scaled by WH_SCALE for fp8 weights):
     D_t   = beta*D_{t-1} + (xp'_t + spk_{t-1} @ Wh'^T)
     mem_t = alpha*mem_{t-1} + sum_br D_t - s*spk_{t-1}
     spk_t = (mem_t > s)                      (s = WH_SCALE if fp8 else 1)
  xp'_t = x_t @ Wx'^T + b'  precomputed on-device for all t (bias via x-row==1).
  readout (transposed layout [O, BL], per-step; softmax DEFERRED):
     mem_roT_t = aro*mem_roT_{t-1} + Wro' @ spk_t + bro'
     store mem_roT_t (bf16) into a history buffer
  end-pass: out = sum_{t>10} softmax(mem_roT_t) computed chunk-wise with
  PE column-sums + PE broadcast of 1/sum, all off the critical scan path.

Device layouts (per core, BL=32 batch rows):
  f' blocks jf=0..31 (f' = jf*128+p), h blocks jh=0..7 (h = jh*128+p)
  d    SBUF [128, jf*32+b] f32      mem/spk SBUF [128, jh*32+b]
  whT  SBUF [128, (jhk*32+jf)*128+m] fp8e4m3 x 2^13 (lhsT tiles)
  xp   DRAM [128, chunk-major: c*(JF*512) + jf*512 + (s*32+b)] bf16
       (chunk = 16 timesteps; phase-1 stores and scan loads are contiguous)
"""

import numpy as np
import ml_dtypes

import concourse.bass as bass
import concourse.bacc as bacc
import concourse.mybir as mybir
import concourse.tile as tile
from concourse.bass_utils import run_bass_kernel_spmd

F32 = mybir.dt.float32
BF16 = mybir.dt.bfloat16
FP8 = mybir.dt.float8e4

B, T_FULL, IN_DIM = 256, 500, 700
H, BR, O = 1024, 4, 20
NCORES = 8
BL = B // NCORES            # 32 batch rows per core
KT = 6                      # k-tiles for input dim (700 + bias row -> 768)
KIN = KT * 128              # 768
JF = (H * BR) // 128        # 32 feature blocks
JH = H // 128               # 8 hidden blocks
CH = 16                     # timesteps per xp chunk (512 cols = 1 psum bank)
NCH = CH * BL               # 512 columns per chunk
CHW = JF * NCH              # xp cols per chunk (16384)
WARMUP = 10
WH_SCALE = 8192.0           # 2**13 centers Wh in fp8e4m3's normal range


def _sigmoid(x):
    return 1.0 / (1.0 + np.exp(-x))


def _bf(a):
    return np.ascontiguousarray(a.astype(ml_dtypes.bfloat16))


def _f32(a):
    return np.ascontiguousarray(a.astype(np.float32))


def _fp8(a):
    return np.ascontiguousarray(
        np.clip(a, -448.0, 448.0).astype(ml_dtypes.float8_e4m3))


def prepare_inputs(x, W_dense, b_dense, tau_n, tau_m, W_ro, b_ro, tau_m_ro,
                   wh_fp8=True):
    x = np.asarray(x, np.float32)
    W = np.asarray(W_dense, np.float32)
    b = np.asarray(b_dense, np.float32)
    beta = _sigmoid(np.asarray(tau_n, np.float32))      # [H, BR]
    alpha = _sigmoid(np.asarray(tau_m, np.float32))     # [H]
    aro = _sigmoid(np.asarray(tau_m_ro, np.float32))    # [O]
    W_ro = np.asarray(W_ro, np.float32)
    b_ro = np.asarray(b_ro, np.float32)

    # branch-major permutation f' = br*H + h  (row f = h*BR + br)
    brs, hs = np.meshgrid(np.arange(BR), np.arange(H), indexing="ij")
    perm = (hs * BR + brs).reshape(-1)
    Wp = W[perm]                                         # [4096, 1724]
    bp = b[perm]
    beta_f = beta.T.reshape(-1)                          # beta[f'=br*H+h]
    alpha_f = np.tile(alpha, BR)                         # alpha[h] per f'
    c = (1.0 - alpha_f) * (1.0 - beta_f)

    Wx = c[:, None] * Wp[:, :IN_DIM]                     # [4096, 700]
    Wh = c[:, None] * Wp[:, IN_DIM:]                     # [4096, 1024]
    bp = c * bp

    if wh_fp8:
        Wx = Wx * WH_SCALE
        Wh = Wh * WH_SCALE
        bp = bp * WH_SCALE

    Wx_aug = np.zeros((H * BR, KIN), np.float32)
    Wx_aug[:, :IN_DIM] = Wx
    Wx_aug[:, IN_DIM] = bp                               # bias via x-row == 1

    # lhsT packs: [p, (kt|jhk, jf), m] with lhsT[p, m] = W[jf*128+m, kt*128+p]
    wxT = Wx_aug.reshape(JF, 128, KT, 128).transpose(3, 2, 0, 1).reshape(128, KT * JF * 128)
    whT = Wh.reshape(JF, 128, JH, 128).transpose(3, 2, 0, 1).reshape(128, JH * JF * 128)

    beta_sb = np.repeat(beta_f.reshape(JF, 128).T[:, :, None], BL, axis=2).reshape(128, JF * BL)
    alpha_sb = np.repeat(alpha.reshape(JH, 128).T[:, :, None], BL, axis=2).reshape(128, JH * BL)

    Wrop = (1.0 - aro)[:, None] * W_ro                   # [O, H]
    brop = (1.0 - aro) * b_ro
    wroT = Wrop.reshape(O, JH, 128).transpose(2, 1, 0).reshape(128, JH * O)

    common = {
        "whT": _fp8(whT) if wh_fp8 else _bf(whT),
        "wxT": _bf(wxT),
        "beta": _f32(beta_sb),
        "alpha": _f32(alpha_sb),
        "wro": _bf(wroT),
        "bro": _bf(brop.reshape(1, O)),
        "aroc": _f32(aro.reshape(O, 1)),
    }

    n_chk = (x.shape[1] * BL + NCH - 1) // NCH
    in_maps = []
    for core in range(NCORES):
        xc = x[core * BL:(core + 1) * BL]                # [32, T, 700]
        t_len = xc.shape[1]
        xT = np.zeros((KIN, n_chk * NCH), np.float32)    # zero pad past t_len
        xT[:IN_DIM, :t_len * BL] = xc.transpose(2, 1, 0).reshape(IN_DIM, t_len * BL)
        xT[IN_DIM, :t_len * BL] = 1.0
        m = dict(common)
        m["xT"] = _bf(xT)
        in_maps.append(m)
    return in_maps


def build_module(t_len=T_FULL, repeat=1, pre_mult=1, scan_mult=1,
                 phases="both", wh_fp8=True, readout=True, pool_ops=False):
    # scan structure: chunk 0 peeled, steady chunk pairs, short tail chunk
    n_chk = (t_len * BL + NCH - 1) // NCH                # xp chunks (32 for T=500)
    tail_steps = t_len - (n_chk - 1) * CH                # steps in last chunk
    assert n_chk >= 4 and (n_chk - 2) % 2 == 0, \
        "steady loop needs an even number of full chunks after the peel"

    nc = bacc.Bacc("TRN2", target_bir_lowering=False, debug=False)

    xT = nc.dram_tensor("xT", [KIN, n_chk * NCH], BF16, kind="ExternalInput").ap()
    whT = nc.dram_tensor("whT", [128, JH * JF * 128],
                         FP8 if wh_fp8 else BF16, kind="ExternalInput").ap()
    wxT = nc.dram_tensor("wxT", [128, KT * JF * 128], BF16, kind="ExternalInput").ap()
    beta_in = nc.dram_tensor("beta", [128, JF * BL], F32, kind="ExternalInput").ap()
    alpha_in = nc.dram_tensor("alpha", [128, JH * BL], F32, kind="ExternalInput").ap()
    wro_in = nc.dram_tensor("wro", [128, JH * O], BF16, kind="ExternalInput").ap()
    bro_in = nc.dram_tensor("bro", [1, O], BF16, kind="ExternalInput").ap()
    aroc_in = nc.dram_tensor("aroc", [O, 1], F32, kind="ExternalInput").ap()
    out = nc.dram_tensor("out", [O, BL], F32, kind="ExternalOutput").ap()
    xp = nc.dram_tensor("xp", [128, n_chk * CHW], BF16).ap()
    hist = nc.dram_tensor("hist", [O, n_chk * CHW], BF16).ap()

    with tile.TileContext(nc) as tc:
        _emit(tc, xT, whT, wxT, beta_in, alpha_in, wro_in, bro_in, aroc_in,
              out, xp, hist, n_chk=n_chk, tail_steps=tail_steps, repeat=repeat,
              pre_mult=pre_mult, scan_mult=scan_mult, phases=phases,
              wh_fp8=wh_fp8, readout=readout, pool_ops=pool_ops)
    nc.compile()
    return nc


def _emit(tc, xT, whT, wxT, beta_in, alpha_in, wro_in, bro_in, aroc_in,
          out, xp, hist, n_chk, tail_steps, repeat=1, pre_mult=1, scan_mult=1,
          phases="both", wh_fp8=True, readout=True, pool_ops=False):
    nc = tc.nc
    do_pre = phases in ("both", "pre")
    do_scan = phases in ("both", "scan")
    vth = WH_SCALE if wh_fp8 else 1.0

    SCR = 2 * CHW                                        # scratch cols (bf16)

    with (
        tc.tile_pool(name="const", bufs=1) as cpool,
        tc.tile_pool(name="state", bufs=1) as spool,
        tc.tile_pool(name="sm", bufs=2) as smp,
        tc.tile_pool(name="mmps", bufs=6, space="PSUM") as mmps,
        tc.tile_pool(name="rops", bufs=2, space="PSUM") as rops,
    ):
        # ---- resident constants ----
        wx_sb = cpool.tile([128, KT * JF * 128], BF16, tag="wx")
        wh_sb = cpool.tile([128, JH * JF * 128], FP8 if wh_fp8 else BF16, tag="wh")
        beta_sb = cpool.tile([128, JF * BL], F32, tag="beta")
        alpha_sb = cpool.tile([128, JH * BL], F32, tag="alpha")
        wro_sb = cpool.tile([128, JH * O], BF16, tag="wro")
        bro_sb = cpool.tile([1, O], BF16, tag="bro")
        aroc_sb = cpool.tile([O, 1], F32, tag="aroc")
        ones_sb = cpool.tile([1, BL], BF16, tag="ones")
        ones20 = cpool.tile([O, 1], F32, tag="ones20")
        onesr = cpool.tile([1, O], F32, tag="onesr")
        nc.sync.dma_start(wx_sb[:], wxT[:])
        nc.sync.dma_start(wh_sb[:], whT[:])
        nc.sync.dma_start(beta_sb[:], beta_in[:])
        nc.sync.dma_start(alpha_sb[:], alpha_in[:])
        nc.sync.dma_start(wro_sb[:], wro_in[:])
        nc.sync.dma_start(bro_sb[:], bro_in[:])
        nc.sync.dma_start(aroc_sb[:], aroc_in[:])
        nc.vector.memset(ones_sb[:], 1.0)
        nc.vector.memset(ones20[:], 1.0)
        nc.vector.memset(onesr[:], 1.0)

        # ---- scratch: phase-1 staging / scan chunk buffers / end-pass ----
        scratch = spool.tile([128, SCR], BF16, tag="scratch")
        xin = [[scratch[:, (i * KT + kt) * NCH:(i * KT + kt + 1) * NCH]
                for kt in range(KT)] for i in range(2)]
        evb = [scratch[:, 2 * KT * NCH + i * 8 * NCH:
                       2 * KT * NCH + (i + 1) * 8 * NCH] for i in range(2)]
        xpc = [scratch[:, i * CHW:(i + 1) * CHW] for i in range(2)]

        # ---- persistent state ----
        d = spool.tile([128, JF * BL], F32, tag="d")
        mem = spool.tile([128, JH * BL], F32, tag="mem")
        spk = [spool.tile([128, JH * BL], BF16, tag=f"spk{i}", name=f"spk{i}")
               for i in range(2)]
        spkb = ([spool.tile([128, JH * BL], F32, tag=f"spkb{i}", name=f"spkb{i}")
                 for i in range(2)] if wh_fp8 else spk)
        mem_roT = spool.tile([O, BL], F32, tag="mrt")
        accT = spool.tile([O, BL], F32, tag="accT")
        histc = [spool.tile([O, NCH], BF16, tag=f"hc{i}", name=f"hc{i}")
                 for i in range(2)]
        l_t1 = spool.tile([128, 4 * BL], F32, tag="lt1")
        l_t2 = spool.tile([128, 4 * BL], F32, tag="lt2")
        l_half = spool.tile([128, 4 * BL], F32, tag="lh")

        # ================= phase 1: xp = x @ Wx'^T =================
        def load_x(buf, c):
            for kt in range(KT):
                nc.sync.dma_start(xin[buf][kt],
                                  xT[kt * 128:(kt + 1) * 128, c * NCH:(c + 1) * NCH])

        def pre_chunk(buf, c):
            for g in range(4):                           # 4 groups x 8 jf-blocks
                ev = evb[g % 2]
                for jl in range(8):
                    jf = g * 8 + jl
                    ps = mmps.tile([128, 512], F32, tag="mm")
                    for kt in range(KT):
                        nc.tensor.matmul(
                            ps[:],
                            wx_sb[:, (kt * JF + jf) * 128:(kt * JF + jf + 1) * 128],
                            xin[buf][kt],
                            start=(kt == 0), stop=(kt == KT - 1),
                        )
                    nc.scalar.copy(ev[:, jl * NCH:(jl + 1) * NCH], ps[:])
                nc.scalar.dma_start(
                    xp[:, c * CHW + g * 8 * NCH: c * CHW + (g + 1) * 8 * NCH],
                    ev)

        # ================= phase 2: the scan =================
        def load_chunk(buf, xoff):
            """DMA xp chunk (16 steps, contiguous CHW cols) into xpc[buf]."""
            src = (xp[:, bass.ds(xoff, CHW)] if not isinstance(xoff, int)
                   else xp[:, xoff:xoff + CHW])
            nc.sync.dma_start(xpc[buf], src)

        d_v = d[:].rearrange("p (br c) -> p br c", br=BR)
        beta_v = beta_sb[:].rearrange("p (br c) -> p br c", br=BR)

        def emit_step(buf, s, par):
            """One timestep: s = index within chunk, par = parity of global t."""
            spk_prev, spk_cur = spk[par ^ 1], spk[par]
            xpc_v = xpc[buf].rearrange("p (br q s b) -> p br q s b",
                                       br=BR, q=JH, s=CH)
            hps = []
            for half in (0, 1):
                hp = mmps.tile([128, 512], F32, tag="mm")
                for bri in range(BR):
                    for jho in range(4):
                        jf = bri * 8 + half * 4 + jho
                        o_sl = hp[:, bri * 128 + jho * 32: bri * 128 + jho * 32 + 32]
                        for jhk in range(JH):
                            nc.tensor.matmul(
                                o_sl,
                                wh_sb[:, (jhk * JF + jf) * 128:(jhk * JF + jf + 1) * 128],
                                spk_prev[:, jhk * 32:jhk * 32 + 32],
                                start=(jhk == 0), stop=(jhk == JH - 1),
                            )
                hps.append(hp)

            for half in (0, 1):
                hp = hps[half]
                off = half * 128
                dsl = d_v[:, :, off:off + 128]
                bsl = beta_v[:, :, off:off + 128]
                xsl = xpc_v[:, :, half * 4:half * 4 + 4, s, :]
                nc.vector.tensor_tensor(dsl, dsl, bsl, mybir.AluOpType.mult)
                nc.vector.tensor_tensor(dsl, dsl, hp[:], mybir.AluOpType.add)
                nc.vector.tensor_tensor(dsl, dsl, xsl, mybir.AluOpType.add)
                # branch sum -> l_half [128, 128] (SBUF-only: Pool-eligible)
                eng = nc.gpsimd if pool_ops else nc.vector
                eng.tensor_tensor(l_t1[:], d_v[:, 0, off:off + 128],
                                  d_v[:, 1, off:off + 128], mybir.AluOpType.add)
                eng.tensor_tensor(l_t2[:], d_v[:, 2, off:off + 128],
                                  d_v[:, 3, off:off + 128], mybir.AluOpType.add)
                eng.tensor_tensor(l_half[:], l_t1[:], l_t2[:], mybir.AluOpType.add)
                # mem update + spike
                msl = mem[:, off:off + 128]
                eng.tensor_tensor(msl, msl, alpha_sb[:, off:off + 128],
                                  mybir.AluOpType.mult)
                eng.tensor_tensor(msl, msl, l_half[:], mybir.AluOpType.add)
                eng.tensor_tensor(msl, msl, spkb[par ^ 1][:, off:off + 128],
                                  mybir.AluOpType.subtract)
                eng.tensor_scalar(spk_cur[:, off:off + 128], msl, vth, None,
                                  mybir.AluOpType.is_gt)
                if wh_fp8:
                    eng.tensor_scalar(spkb[par][:, off:off + 128], msl, vth,
                                      vth, mybir.AluOpType.is_gt,
                                      mybir.AluOpType.mult)

            # ---- readout: transposed [O, BL]; softmax deferred to end-pass ----
            if not readout:
                return
            roT = rops.tile([O, BL], F32, tag="ro")
            for jh in range(JH):
                nc.tensor.matmul(
                    roT[:], wro_sb[:, jh * O:(jh + 1) * O],
                    spk_cur[:, jh * 32:jh * 32 + 32],
                    start=(jh == 0), stop=False,
                )
            nc.tensor.matmul(roT[:], bro_sb[:], ones_sb[:], start=False, stop=True)
            nc.scalar.activation(mem_roT[:], mem_roT[:],
                                 mybir.ActivationFunctionType.Copy,
                                 scale=aroc_sb[:, 0:1])
            nc.vector.tensor_tensor(mem_roT[:], mem_roT[:], roT[:],
                                    mybir.AluOpType.add)
            nc.scalar.copy(histc[buf][:, s * BL:(s + 1) * BL], mem_roT[:])

        def flush_hist(buf, xoff, ncols=NCH):
            if not readout:
                return
            dst = (hist[:, bass.ds(xoff, ncols)] if not isinstance(xoff, int)
                   else hist[:, xoff:xoff + ncols])
            nc.scalar.dma_start(dst, histc[buf][:, :ncols])

        for _rep in range(repeat):
            # ---- reset state ----
            nc.vector.memset(d[:], 0.0)
            nc.vector.memset(mem[:], 0.0)
            nc.vector.memset(spk[0][:], 0.0)
            nc.vector.memset(spk[1][:], 0.0)
            if wh_fp8:
                nc.vector.memset(spkb[0][:], 0.0)
                nc.vector.memset(spkb[1][:], 0.0)
            nc.vector.memset(mem_roT[:], 0.0)
            nc.vector.memset(accT[:], 0.0)

            # ---- phase 1 (fully unrolled; loads on qSP, stores on qAct) ----
            if do_pre:
                for _pm in range(pre_mult):
                    load_x(0, 0)
                    for c in range(n_chk):
                        if c + 1 < n_chk:
                            load_x((c + 1) % 2, c + 1)
                        pre_chunk(c % 2, c)

            tc.strict_bb_all_engine_barrier()

            # ---- phase 2: scan ----
            if do_scan:
                for _sm in range(scan_mult):
                    # peel: chunk 0 (t = 0..CH-1), prefetch chunk 1
                    load_chunk(0, 0)
                    load_chunk(1, CHW)
                    for s in range(CH):
                        emit_step(0, s, s & 1)
                    flush_hist(0, 0)

                    # steady: chunk pairs (1,2), (3,4), ... (n_chk-3, n_chk-2)
                    with tc.For_i(CHW, (n_chk - 2) * CHW, 2 * CHW,
                                  hint_engines=(mybir.EngineType.PE,)) as xoff:
                        load_chunk(0, xoff + CHW)
                        for s in range(CH):
                            emit_step(1, s, s & 1)
                        flush_hist(1, xoff)
                        load_chunk(1, xoff + 2 * CHW)     # prefetch (last: tail chunk)
                        for s in range(CH):
                            emit_step(0, s, s & 1)
                        flush_hist(0, xoff + CHW)

                    # tail: chunk n_chk-1 (already in xpc[1]); flush all NCH
                    # cols (stale cols stay finite and are excluded from the
                    # end-pass reduce)
                    for s in range(tail_steps):
                        emit_step(1, s, s & 1)
                    flush_hist(1, (n_chk - 1) * CHW)

            # ---- end-pass: out = sum_{t>WARMUP} softmax(mem_roT_t) ----
            if do_scan and readout:
                hist_sb = scratch[0:O, 0:n_chk * NCH]
                hist_v = hist_sb.rearrange("p (c n) -> p c n", c=n_chk)
                src = hist.rearrange("p (c w) -> p c w", c=n_chk)
                nc.sync.dma_start(hist_v, src[:, :, 0:NCH])
                for c in range(n_chk):
                    t0 = c * CH
                    lo = max(WARMUP + 1 - t0, 0)          # skip warmup steps
                    hi = tail_steps if c == n_chk - 1 else CH
                    if lo >= hi:
                        continue
                    hv = hist_v[:, c, :]
                    e = smp.tile([O, NCH], F32, tag="e")
                    nc.scalar.activation(e[:], hv, mybir.ActivationFunctionType.Exp)
                    sp = mmps.tile([1, 512], F32, tag="mm")
                    nc.tensor.matmul(sp[:, :NCH], ones20[:], e[:], start=True,
                                     stop=True)
                    rp = smp.tile([1, NCH], F32, tag="rp")
                    nc.vector.reciprocal(rp[:], sp[:, :NCH])
                    bc = mmps.tile([128, 512], F32, tag="mm")
                    nc.tensor.matmul(bc[:O, :NCH], onesr[:], rp[:], start=True,
                                     stop=True)
                    pr = smp.tile([O, NCH], F32, tag="pr")
                    nc.vector.tensor_tensor(pr[:], e[:], bc[:O, :NCH],
                                            mybir.AluOpType.mult)
                    pr_v = pr[:].rearrange("p (s b) -> p b s", s=CH)
                    red = smp.tile([O, BL], F32, tag="red")
                    nc.vector.tensor_reduce(red[:], pr_v[:, :, lo:hi],
                                            mybir.AxisListType.X,
                                            mybir.AluOpType.add)
                    nc.vector.tensor_tensor(accT[:], accT[:], red[:],
                                            mybir.AluOpType.add)

            nc.sync.dma_start(out[:], accT[:])


_NC_CACHE = {}


def _get_module(t_len):
    if t_len not in _NC_CACHE:
        _NC_CACHE[t_len] = build_module(t_len)
    return _NC_CACHE[t_len]


def run(inputs, trace=False):
    in_maps = prepare_inputs(**inputs)
    t_len = np.asarray(inputs["x"]).shape[1]
    nc = _get_module(t_len)
    res = run_bass_kernel_spmd(nc, in_maps, list(range(NCORES)), trace=trace)
    outs = [res.results[i]["out"].T for i in range(NCORES)]   # [O,BL] -> [BL,O]
    return np.concatenate(outs, axis=0).astype(np.float32), res


def kernel(x, W_dense, b_dense, tau_n, tau_m, W_ro, b_ro, tau_m_ro):
    out, _ = run(dict(x=x, W_dense=W_dense, b_dense=b_dense, tau_n=tau_n,
                      tau_m=tau_m, W_ro=W_ro, b_ro=b_ro, tau_m_ro=tau_m_ro))
    return out


def make_bench(inputs, nc=None, prep_kwargs=None):
    """Build a timed runner with device-resident inputs (for test.py only).

    Mirrors bass2jax.run_bass_via_pjrt's multi-core path, but device_puts the
    inputs once so repeated calls measure device execution, not host transfer.
    """
    import jax
    import numpy as np_
    from jax.sharding import Mesh, PartitionSpec, NamedSharding
    from jax.experimental.shard_map import shard_map
    import concourse.mybir as mybir_
    from concourse import bass2jax

    in_maps = prepare_inputs(**inputs, **(prep_kwargs or {}))
    t_len = np_.asarray(inputs["x"]).shape[1]
    if nc is None:
        nc = _get_module(t_len)
    bass2jax.install_neuronx_cc_hook()

    partition_name = nc.partition_id_tensor.name if nc.partition_id_tensor else None
    in_names, out_names, out_avals, zero_outs = [], [], [], []
    for alloc in nc.m.functions[0].allocations:
        if not isinstance(alloc, mybir_.MemoryLocationSet):
            continue
        name = alloc.memorylocations[0].name
        if alloc.kind == "ExternalInput":
            if name != partition_name:
                in_names.append(name)
        elif alloc.kind == "ExternalOutput":
            shape = tuple(alloc.tensor_shape)
            dtype = mybir_.dt.np(alloc.dtype)
            out_names.append(name)
            out_avals.append(jax.core.ShapedArray(shape, dtype))
            zero_outs.append(np_.zeros(shape, dtype))
    n_params = len(in_names)
    all_in_names = in_names + out_names
    if partition_name is not None:
        all_in_names.append(partition_name)
    donate = tuple(range(n_params, n_params + len(out_names)))

    def _body(*args):
        operands = list(args)
        if partition_name is not None:
            operands.append(bass2jax.partition_id_tensor())
        outs = bass2jax._bass_exec_p.bind(
            *operands,
            out_avals=tuple(out_avals),
            in_names=tuple(all_in_names),
            out_names=tuple(out_names),
            lowering_input_output_aliases=(),
            sim_require_finite=True,
            sim_require_nnan=True,
            nc=nc,
        )
        return tuple(outs)

    devices = jax.devices()[:NCORES]
    mesh = Mesh(np_.asarray(devices), ("core",))
    in_specs = (PartitionSpec("core"),) * (n_params + len(out_names))
    out_specs = (PartitionSpec("core"),) * len(out_names)
    sharded = jax.jit(
        shard_map(_body, mesh=mesh, in_specs=in_specs, out_specs=out_specs,
                  check_rep=False),
        donate_argnums=donate, keep_unused=True,
    )
    concat_in = [
        np_.concatenate([np_.asarray(in_maps[c][name]) for c in range(NCORES)], axis=0)
        for name in in_names
    ]
    sh = NamedSharding(mesh, PartitionSpec("core"))
    dev_in = [jax.device_put(a, sh) for a in concat_in]

    def call():
        zeros = [np_.zeros((NCORES * z.shape[0], *z.shape[1:]), z.dtype)
                 for z in zero_outs]
        outs = sharded(*dev_in, *zeros)
        jax.block_until_ready(outs)
        return outs

    return call
